# revision 9
# baseline (speedup 1.0000x reference)
"""Chamfer distance kernel for Trainium2, 8 NeuronCores.

Math: dist2[m, n] = |y_m|^2 + |x_n|^2 - 2 y_m.x_n, computed as ONE K=16
matmul per tile using a bf16 hi/lo split of every operand (all 4 cross
terms kept), accumulated in fp32 PSUM -> ~1e-5 relative accuracy.
min(sqrt(d)) == sqrt(min(d)), so all mins run on squared distances and the
sqrt happens on the host over just B*(M+N) values.

Sharding: core c handles batch b = c//2, y-half h = c%2 (2048 of 4096 y
rows), all 4096 x rows.  Single pass over D2 tiles [128 m, 2048 n]:
  rowmin: vector.tensor_reduce(min) over the free (n) axis per tile ->
      one [128, 1] slot per tile (host min-combines the 2 per m-block)
  colmin: running vector.tensor_tensor(min) into an SBUF accumulator
      [128, 4096]; lane p holds min over m = i*128+p.  The accumulator is
      DMA'd out and the 128-lane + core-half reduction happens on host.

The 4 matmuls filling one PSUM tile go to 4 distinct 32-row PE groups
(tile_position), so they run concurrently despite K=16 using only 16 rows.
"""

import numpy as np
import ml_dtypes

_B, _N, _M, _D = 4, 4096, 4096, 3
_MHALF = _M // 2
_NCORES = 8
_K = 5                   # [ones, -2y0, -2y1, -2y2, |y|^2] x [xnorm, x0, x1, x2, ones]
_BIG = 3.0e38

_cache = {}


def _side_matrices(xb, yb):
    """Return (ya [5, M'], xa [5, N]) fp32 for one (batch, y-half).

    Row pairing (lhsT row k multiplies rhs row k):
      k0: ones_y <-> xnorm;  k1-3: -2*y_d <-> x_d;  k4: ynorm <-> ones_x
    so sum_k ya[k, m] * xa[k, n] = |y_m|^2 + |x_n|^2 - 2 y_m.x_n.
    """
    n = xb.shape[0]
    m = yb.shape[0]
    xb = np.ascontiguousarray(xb, np.float32)
    yb = np.ascontiguousarray(yb, np.float32)
    xnorm = np.einsum("nd,nd->n", xb, xb, dtype=np.float32, optimize=True)
    ynorm = np.einsum("md,md->m", yb, yb, dtype=np.float32, optimize=True)
    ya = np.empty((_K, m), np.float32)
    xa = np.empty((_K, n), np.float32)
    ya[0] = 1.0
    ya[1:4] = -2.0 * yb.T
    ya[4] = ynorm
    xa[0] = xnorm
    xa[1:4] = xb.T
    xa[4] = 1.0
    return np.ascontiguousarray(ya), np.ascontiguousarray(xa)


def _split_excess_waits(nc, mybir, maxw=1):
    """This walrus build accepts only one sync-wait per instruction; hoist
    extra waits onto dedicated wait-only Drain instructions inserted just
    before the over-limit instruction on the same engine."""
    n_split = 0
    for f in nc.m.functions:
        for b in f.blocks:
            il = b.instructions
            idx = 0
            while idx < len(il):
                ins = il[idx]
                si = ins.sync_info
                if si is not None and len(si.on_wait) > maxw:
                    waits = list(si.on_wait)
                    keep = waits[-maxw:]
                    extra = waits[:-maxw]
                    ins.sync_info = mybir.SyncInfo(
                        on_wait=keep, on_update=list(si.on_update)
                    )
                    for j in range(0, len(extra), maxw):
                        d = mybir.InstDrain(
                            name=f"{ins.name}-wsplit{j}",
                            engine=ins.engine,
                            ins=[],
                            outs=[],
                            sync_info=mybir.SyncInfo(
                                on_wait=extra[j : j + maxw], on_update=[]
                            ),
                        )
                        il.insert(idx, d)
                        idx += 1
                    n_split += 1
                idx += 1
    return n_split


def build_bass():
    """Build the single SPMD Bass module (same program on all 8 cores)."""
    import concourse.bass as bass
    import concourse.tile as tile
    from concourse import mybir

    MIN = mybir.AluOpType.min
    f32 = mybir.dt.float32

    nc = bass.Bass(trn_type="TRN2")
    ya_d = nc.dram_tensor("ya", [_K, _MHALF], f32, kind="ExternalInput")
    xa_d = nc.dram_tensor("xa", [_K, _N], f32, kind="ExternalInput")
    n_mblk = _MHALF // 128          # 16
    TW = 2048                       # psum tile free width (4 banks)
    n_tiles_per_blk = _N // TW      # 2
    n_slots = n_mblk * n_tiles_per_blk  # 32 rowmin slots
    rowmin_d = nc.dram_tensor("rowmin", [128, n_slots], f32, kind="ExternalOutput")
    colacc_d = nc.dram_tensor("colacc", [128, _N], f32, kind="ExternalOutput")

    with tile.TileContext(nc) as tc:
        with (
            tc.tile_pool(name="inputs", bufs=1) as inputs,
            tc.tile_pool(name="outs", bufs=1) as outs,
            tc.tile_pool(name="psum", bufs=2, space="PSUM") as psum,
        ):
            # Replicate both sides at partition offsets 0/32/64/96 so four
            # K=5 matmuls can occupy four distinct PE row groups.
            yr = inputs.tile([128, _MHALF], f32)
            xr = inputs.tile([128, _N], f32)
            for g in range(4):
                nc.sync.dma_start(out=yr[32 * g : 32 * g + _K, :], in_=ya_d[:, :])
                nc.sync.dma_start(out=xr[32 * g : 32 * g + _K, :], in_=xa_d[:, :])

            rowmin = outs.tile([128, n_slots], f32)
            colacc = outs.tile([128, _N], f32)

            for i in range(n_mblk):
                for j in range(n_tiles_per_blk):
                    pt = psum.tile([128, TW], f32)
                    for q in range(4):
                        p0 = 32 * q
                        c0 = j * TW + q * 512
                        nc.tensor.matmul(
                            pt[:, q * 512 : (q + 1) * 512],
                            lhsT=yr[p0 : p0 + _K, i * 128 : (i + 1) * 128],
                            rhs=xr[p0 : p0 + _K, c0 : c0 + 512],
                            start=True,
                            stop=True,
                            tile_position=(p0, 0),
                        )
                    slot = i * n_tiles_per_blk + j
                    nc.vector.tensor_reduce(
                        out=rowmin[:, slot : slot + 1],
                        in_=pt[:, :],
                        axis=mybir.AxisListType.X,
                        op=MIN,
                    )
                    aslice = colacc[:, j * TW : (j + 1) * TW]
                    if i == 0:
                        nc.vector.tensor_copy(aslice, pt[:, :])
                    else:
                        nc.vector.tensor_tensor(
                            out=aslice, in0=pt[:, :], in1=aslice, op=MIN
                        )

            nc.sync.dma_start(out=rowmin_d[:, :], in_=rowmin[:, :])
            nc.sync.dma_start(out=colacc_d[:, :], in_=colacc[:, :])

    _split_excess_waits(nc, mybir)
    return nc


def _get_nc():
    if "nc" not in _cache:
        _cache["nc"] = build_bass()
    return _cache["nc"]


def make_in_maps(x, y):
    """Per-core input dicts: core c -> (batch c//2, y-half c%2)."""
    x = np.asarray(x, dtype=np.float32)
    y = np.asarray(y, dtype=np.float32)
    in_maps = []
    for c in range(_NCORES):
        b, h = divmod(c, 2)
        ya, xa = _side_matrices(x[b], y[b, h * _MHALF : (h + 1) * _MHALF])
        in_maps.append({"ya": ya, "xa": xa})
    return in_maps


def reduce_outputs(results):
    """Host-side gather: per-core mins -> final scalar."""
    d2_m = np.empty((_B, _M), np.float64)
    d2_n = np.full((_B, _N), np.inf, np.float64)
    for c, r in enumerate(results):
        b, h = divmod(c, 2)
        rm = np.asarray(r["rowmin"], np.float64)   # [128, 32]; m = i*128 + p
        rm_blk = rm.reshape(128, -1, 2).min(axis=2)   # [128, 16]; 2 n-tiles/blk
        d2_m[b, h * _MHALF : (h + 1) * _MHALF] = rm_blk.T.reshape(-1)
        ca = np.asarray(r["colacc"], np.float64)   # [128, 4096]
        np.minimum(d2_n[b], ca.min(axis=0), out=d2_n[b])
    mean_m = np.sqrt(np.maximum(d2_m, 0.0)).mean()
    mean_n = np.sqrt(np.maximum(d2_n, 0.0)).mean()
    return np.float32(mean_m + mean_n)


def kernel(x, y):
    from concourse.bass_utils import run_bass_kernel_spmd

    nc = _get_nc()
    in_maps = make_in_maps(x, y)
    res = run_bass_kernel_spmd(nc, in_maps, core_ids=list(range(_NCORES)))
    return reduce_outputs(res.results)


# revision 21
# speedup vs baseline: 18.2487x; 18.2487x over previous
"""Chamfer distance kernel for Trainium2, 8 NeuronCores.

Math: dist2[m, n] = |y_m|^2 + |x_n|^2 - 2 y_m.x_n, computed as ONE K=16
matmul per tile using a bf16 hi/lo split of every operand (all 4 cross
terms kept), accumulated in fp32 PSUM -> ~1e-5 relative accuracy.
min(sqrt(d)) == sqrt(min(d)), so all mins run on squared distances and the
sqrt happens on the host over just B*(M+N) values.

Sharding: core c handles batch b = c//2, y-half h = c%2 (2048 of 4096 y
rows), all 4096 x rows.  Single pass over D2 tiles [128 m, 2048 n]:
  rowmin: vector.tensor_reduce(min) over the free (n) axis per tile ->
      one [128, 1] slot per tile (host min-combines the 2 per m-block)
  colmin: running vector.tensor_tensor(min) into an SBUF accumulator
      [128, 4096]; lane p holds min over m = i*128+p.  The accumulator is
      DMA'd out and the 128-lane + core-half reduction happens on host.

The 4 matmuls filling one PSUM tile go to 4 distinct 32-row PE groups
(tile_position), so they run concurrently despite K=16 using only 16 rows.
"""

import numpy as np
import ml_dtypes

_B, _N, _M, _D = 4, 4096, 4096, 3
_MHALF = _M // 2
_NCORES = 8
_K = 24                  # 3-way bf16 split of [ones|norm|(-2y_d)] x [norm|ones|x_d]
_BIG = 3.0e38

_cache = {}


def _bf16_3split(v):
    """fp32 array -> 3 bf16 parts with v ~= p0 + p1 + p2 (24 mantissa bits)."""
    v = v.astype(np.float32)
    a = v.astype(ml_dtypes.bfloat16)
    r = v - a.astype(np.float32)
    b = r.astype(ml_dtypes.bfloat16)
    c = (r - b.astype(np.float32)).astype(ml_dtypes.bfloat16)
    return [a, b, c]


# product split terms (i, j) with i+j <= 2: error floor ~2^-24 per product
_PAIR_IJ = [(0, 0), (0, 1), (1, 0), (0, 2), (2, 0), (1, 1)]


def _side_matrices(xb, yb):
    """Return (ya [24, M'], xa [24, N]) bf16 for one (batch, y-half).

    sum_k ya[k, m] * xa[k, n] ~= |y_m|^2 + |x_n|^2 - 2 y_m.x_n to ~2^-24,
    using a 3-way bf16 split of every operand:
      k0-2 : ones      <-> xnorm parts      k3-5 : ynorm parts <-> ones
      per d: (-2y_d)_i <-> (x_d)_j for (i, j) in _PAIR_IJ
    """
    n = xb.shape[0]
    m = yb.shape[0]
    xb = np.ascontiguousarray(xb, np.float32)
    yb = np.ascontiguousarray(yb, np.float32)
    xnorm = np.einsum("nd,nd->n", xb, xb, dtype=np.float32, optimize=True)
    ynorm = np.einsum("md,md->m", yb, yb, dtype=np.float32, optimize=True)
    t = (-2.0 * yb).astype(np.float32)
    ones_x = np.ones(n, ml_dtypes.bfloat16)
    ones_y = np.ones(m, ml_dtypes.bfloat16)
    ya_rows, xa_rows = [], []
    for part in _bf16_3split(xnorm):
        ya_rows.append(ones_y)
        xa_rows.append(part)
    for part in _bf16_3split(ynorm):
        ya_rows.append(part)
        xa_rows.append(ones_x)
    for d in range(_D):
        ts = _bf16_3split(t[:, d])
        xs = _bf16_3split(xb[:, d])
        for i, j in _PAIR_IJ:
            ya_rows.append(ts[i])
            xa_rows.append(xs[j])
    ya = np.ascontiguousarray(np.stack(ya_rows), dtype=ml_dtypes.bfloat16)
    xa = np.ascontiguousarray(np.stack(xa_rows), dtype=ml_dtypes.bfloat16)
    assert ya.shape[0] == _K
    return ya, xa


def _split_excess_waits(nc, mybir, maxw=1):
    """This walrus build accepts only one sync-wait per instruction; hoist
    extra waits onto wait-only Drain instructions inserted just before the
    over-limit instruction on the same engine.  (A wait-only EventSemaphore
    looks cheaper but wedges the device — empirically it must carry an
    update; Drain is safe.)"""
    n_split = 0
    for f in nc.m.functions:
        for b in f.blocks:
            il = b.instructions
            idx = 0
            while idx < len(il):
                ins = il[idx]
                si = ins.sync_info
                if si is not None and len(si.on_wait) > maxw:
                    waits = list(si.on_wait)
                    keep = waits[-maxw:]
                    extra = waits[:-maxw]
                    ins.sync_info = mybir.SyncInfo(
                        on_wait=keep, on_update=list(si.on_update)
                    )
                    for j in range(0, len(extra), maxw):
                        d = mybir.InstDrain(
                            name=f"{ins.name}-wsplit{j}",
                            engine=ins.engine,
                            ins=[],
                            outs=[],
                            sync_info=mybir.SyncInfo(
                                on_wait=extra[j : j + maxw], on_update=[]
                            ),
                        )
                        il.insert(idx, d)
                        idx += 1
                    n_split += 1
                idx += 1
    return n_split


def build_bass(loop_n=1):
    """Build the single SPMD Bass module (same program on all 8 cores).

    loop_n > 1 wraps the compute body in an on-device For_i that repeats the
    (idempotent) min accumulation — used by test.py to measure the per
    -iteration hardware time without RPC noise."""
    import contextlib
    import concourse.bass as bass
    import concourse.tile as tile
    from concourse import mybir

    MIN = mybir.AluOpType.min
    f32 = mybir.dt.float32
    bf16 = mybir.dt.bfloat16

    nc = bass.Bass(trn_type="TRN2")
    ya_d = nc.dram_tensor("ya", [_K, _MHALF], bf16, kind="ExternalInput")
    xa_d = nc.dram_tensor("xa", [_K, _N], bf16, kind="ExternalInput")
    n_mblk = _MHALF // 128          # 16
    TW = 2048                       # psum tile free width (4 banks)
    n_tiles_per_blk = _N // TW      # 2
    n_slots = n_mblk * n_tiles_per_blk  # 32 rowmin slots
    rowmin_d = nc.dram_tensor("rowmin", [128, n_slots], f32, kind="ExternalOutput")
    colacc_d = nc.dram_tensor("colacc", [128, _N], f32, kind="ExternalOutput")

    with tile.TileContext(nc) as tc:
        with (
            tc.tile_pool(name="inputs", bufs=1) as inputs,
            tc.tile_pool(name="outs", bufs=1) as outs,
            tc.tile_pool(name="psum", bufs=2, space="PSUM") as psum,
        ):
            yr = inputs.tile([128, _MHALF], bf16)
            xr = inputs.tile([128, _N], bf16)
            nc.sync.dma_start(out=yr[:_K, :], in_=ya_d[:, :])
            nc.sync.dma_start(out=xr[:_K, :], in_=xa_d[:, :])

            rowmin = outs.tile([128, n_slots], f32)
            colacc = outs.tile([128, _N], f32)

            loop_cm = contextlib.ExitStack()
            if loop_n > 1:
                loop_cm.enter_context(tc.For_i(0, loop_n, 1))

            for i in range(n_mblk):
                for j in range(n_tiles_per_blk):
                    pt = psum.tile([128, TW], f32)
                    for q in range(4):
                        c0 = j * TW + q * 512
                        nc.tensor.matmul(
                            pt[:, q * 512 : (q + 1) * 512],
                            lhsT=yr[:_K, i * 128 : (i + 1) * 128],
                            rhs=xr[:_K, c0 : c0 + 512],
                            start=True,
                            stop=True,
                        )
                    slot = i * n_tiles_per_blk + j
                    nc.vector.tensor_reduce(
                        out=rowmin[:, slot : slot + 1],
                        in_=pt[:, :],
                        axis=mybir.AxisListType.X,
                        op=MIN,
                    )
                    aslice = colacc[:, j * TW : (j + 1) * TW]
                    if i == 0:
                        nc.vector.tensor_copy(aslice, pt[:, :])
                    else:
                        nc.vector.tensor_tensor(
                            out=aslice, in0=pt[:, :], in1=aslice, op=MIN
                        )

            loop_cm.close()
            nc.sync.dma_start(out=rowmin_d[:, :], in_=rowmin[:, :])
            nc.sync.dma_start(out=colacc_d[:, :], in_=colacc[:, :])

    _split_excess_waits(nc, mybir)
    return nc


def _get_nc():
    if "nc" not in _cache:
        _cache["nc"] = build_bass()
    return _cache["nc"]


def make_in_maps(x, y):
    """Per-core input dicts: core c -> (batch c//2, y-half c%2)."""
    x = np.asarray(x, dtype=np.float32)
    y = np.asarray(y, dtype=np.float32)
    in_maps = []
    for c in range(_NCORES):
        b, h = divmod(c, 2)
        ya, xa = _side_matrices(x[b], y[b, h * _MHALF : (h + 1) * _MHALF])
        in_maps.append({"ya": ya, "xa": xa})
    return in_maps


def reduce_outputs(results):
    """Host-side gather: per-core mins -> final scalar."""
    d2_m = np.empty((_B, _M), np.float64)
    d2_n = np.full((_B, _N), np.inf, np.float64)
    for c, r in enumerate(results):
        b, h = divmod(c, 2)
        rm = np.asarray(r["rowmin"], np.float64)   # [128, 32]; m = i*128 + p
        rm_blk = rm.reshape(128, -1, 2).min(axis=2)   # [128, 16]; 2 n-tiles/blk
        d2_m[b, h * _MHALF : (h + 1) * _MHALF] = rm_blk.T.reshape(-1)
        ca = np.asarray(r["colacc"], np.float64)   # [128, 4096]
        np.minimum(d2_n[b], ca.min(axis=0), out=d2_n[b])
    mean_m = np.sqrt(np.maximum(d2_m, 0.0)).mean()
    mean_n = np.sqrt(np.maximum(d2_n, 0.0)).mean()
    return np.float32(mean_m + mean_n)


def kernel(x, y):
    from concourse.bass_utils import run_bass_kernel_spmd

    nc = _get_nc()
    in_maps = make_in_maps(x, y)
    res = run_bass_kernel_spmd(nc, in_maps, core_ids=list(range(_NCORES)))
    return reduce_outputs(res.results)
